# revision 1
# baseline (speedup 1.0000x reference)
"""AttnBlock (GroupNorm + 1-head spatial self-attention + residual) on 8 trn2 cores.

Sharding: B=4 images, 2 cores per image. Each core receives its full image
(GN stats and K/V need all n=4096 positions) and computes the attention rows
for its half of the query positions. Odd cores receive the image rolled by
2048 along n so every core runs the identical SPMD program (attention output
is invariant to a permutation of key positions).

Per core (C=256 split into 2 chunks of 128 partitions):
  GN stats (ACT square-accum + DVE reduces + tiny grouping matmuls) are folded
  into the projection weights: Wq' = Wq*scale_c, bias' = W@shift + b, so x
  feeds every matmul directly (no normalized copy of x is materialized).
  q = Wq'.T@x (cols 0:2048) ; k = Wk'.T@x ; vT = x.T@Wv'
  scoresT[j,i] = k.T q  (transposed: softmax sums land on the matmul K axis)
  e = exp(scoresT/16) on ACT straight from PSUM (no max subtraction: scores
  are ~N(0,1), exp never overflows fp32)
  den[i] = sum_j e[j,i]: strided reduces + one ones-vector matmul
  AV: h_unnorm[c,i] = sum_j vT[j,c] e[j,i] ; O_unnorm = Wo.T @ h_unnorm
  Device returns O_unnorm and den; the host computes
  out = x + O_unnorm/den + bo  (normalization commutes with the 1x1 conv),
  keeping the residual in exact fp32.
All matmuls run as float32r (tf32-style rounded fp32; ~1e-5 rel precision,
1 cycle/row streaming).
"""

import numpy as np

N = 4096  # spatial positions per image
NHALF = 2048  # query positions per core
C = 256
NCHUNK = 2  # channel chunks of 128
P = 128
NG = 32  # groups
GS = 8  # channels per group
EPS = 1e-6
SCALE = float(C) ** -0.5  # 0.0625
NBLK = 4  # i-blocks of 512 per core
BLK = 512
NJC = 32  # j-chunks of 128
QUART = 4  # j-chunks per exp quarter-buffer
DEN_ENGINE = "gpsimd"  # or "vector"

_CACHE = {}


def _build_program():
    import concourse.bacc as bacc
    import concourse.mybir as mybir
    import concourse.tile as tile

    f32 = mybir.dt.float32
    f32r = mybir.dt.float32r
    AF = mybir.ActivationFunctionType
    OP = mybir.AluOpType
    AX = mybir.AxisListType

    nc = bacc.Bacc("TRN2", target_bir_lowering=False)

    # DRAM I/O
    xa_d = nc.dram_tensor("xa", [NCHUNK, P, NHALF], f32r, kind="ExternalInput")
    xb_d = nc.dram_tensor("xb", [NCHUNK, P, NHALF], f32r, kind="ExternalInput")
    wq_d = nc.dram_tensor("wq", [P, NCHUNK, NCHUNK, P], f32r, kind="ExternalInput")
    wo_d = nc.dram_tensor("wo", [P, NCHUNK, NCHUNK, P], f32r, kind="ExternalInput")
    wv_d = nc.dram_tensor("wv", [P, NCHUNK, C], f32r, kind="ExternalInput")
    bq_d = nc.dram_tensor("bq", [P, NCHUNK], f32, kind="ExternalInput")
    out_d = nc.dram_tensor("out", [NCHUNK, P, NHALF], f32, kind="ExternalOutput")
    den_d = nc.dram_tensor("den", [1, NHALF], f32, kind="ExternalOutput")

    with tile.TileContext(nc) as tc:
        den_eng = nc.gpsimd if DEN_ENGINE == "gpsimd" else nc.vector
        with (
            tc.tile_pool(name="res", bufs=1) as res_pool,
            tc.tile_pool(name="big16", bufs=4) as big16_pool,
            tc.tile_pool(name="rpool", bufs=1) as r_pool,
            tc.tile_pool(name="vpool", bufs=1) as v_pool,
            tc.tile_pool(name="hpool", bufs=2) as h_pool,
            tc.tile_pool(name="opool", bufs=3) as o_pool,
            tc.tile_pool(name="wpool", bufs=1) as w_pool,
            tc.tile_pool(name="small", bufs=1) as s_pool,
            tc.tile_pool(name="scr", bufs=2) as scr_pool,
            tc.tile_pool(name="ps_s", bufs=2, space="PSUM") as ps_s,
            tc.tile_pool(name="ps_av", bufs=1, space="PSUM") as ps_av,
            tc.tile_pool(name="ps_misc", bufs=2, space="PSUM") as ps_misc,
        ):
            # ---- loads ----
            # biases (tiny) + q/k/v weights on sync; xa gates block-0 scores
            # (q needs all of it) so it is split between the scalar queue and
            # sync right behind the weights; xb streams on the gpsimd SWDGE
            # queue; wo goes last (first needed at block-0 output projection).
            bq2 = s_pool.tile([P, NCHUNK], f32, tag="bq")
            nc.sync.dma_start(bq2[:], bq_d.ap())

            wq = w_pool.tile([P, NCHUNK, NCHUNK, P], f32r, tag="wq")
            nc.sync.dma_start(wq[:], wq_d.ap())
            wv = w_pool.tile([P, NCHUNK, C], f32r, tag="wv")
            nc.sync.dma_start(wv[:], wv_d.ap())

            xa = res_pool.tile([P, NCHUNK, NHALF], f32r, tag="xa")
            xb = res_pool.tile([P, NCHUNK, NHALF], f32r, tag="xb")
            for h4 in range(2):
                sl = slice(h4 * BLK, (h4 + 1) * BLK)
                nc.scalar.dma_start(
                    xa[:, :, sl], xa_d.ap().rearrange("a p n -> p a n")[:, :, sl]
                )
            for h4 in range(2, 4):
                sl = slice(h4 * BLK, (h4 + 1) * BLK)
                nc.sync.dma_start(
                    xa[:, :, sl], xa_d.ap().rearrange("a p n -> p a n")[:, :, sl]
                )
            for h4 in range(4):
                sl = slice(h4 * BLK, (h4 + 1) * BLK)
                nc.gpsimd.dma_start(
                    xb[:, :, sl], xb_d.ap().rearrange("a p n -> p a n")[:, :, sl]
                )

            wo = w_pool.tile([P, NCHUNK, NCHUNK, P], f32r, tag="wo")
            nc.scalar.dma_start(wo[:], wo_d.ap())

            ones_c = s_pool.tile([P, 1], f32r, tag="ones_c")
            nc.gpsimd.memset(ones_c[:].bitcast(f32), 1.0)
            zb = s_pool.tile([P, 1], f32, tag="zb")
            nc.gpsimd.memset(zb[:], 0.0)

            vt = v_pool.tile([P, NJC, C], f32r, tag="vt")
            r_t = r_pool.tile([P, NCHUNK, NHALF], f32r, tag="r")

            # ---- projections straight from x ----
            for s in range(8):
                xsrc = xa if s < 4 else xb
                soff = (s % 4) * BLK
                xs0 = xsrc[:, 0, soff : soff + BLK]
                xs1 = xsrc[:, 1, soff : soff + BLK]
                # r = (Wq'^T Wk')^T x + Wk'^T bq', host-precomputed as wq/bq.
                # Neither q nor k is materialized: bk cancels in softmax and
                # q only ever enters the scores through r.
                if s < 4:
                    for b in range(NCHUNK):
                        rp = ps_s.tile([P, BLK], f32, tag="ps_sp")
                        nc.tensor.matmul(
                            rp[:], wq[:, 0, b, :], xs0, start=True, stop=False
                        )
                        nc.tensor.matmul(
                            rp[:], wq[:, 1, b, :], xs1, start=False, stop=True
                        )
                        with nc.allow_low_precision(reason="f32r r"):
                            nc.vector.tensor_scalar_add(
                                r_t[:, b, s * BLK : (s + 1) * BLK],
                                rp[:],
                                bq2[:, b : b + 1],
                            )
                # vT projection: strip s covers j-chunks 4s..4s+3
                for jj in range(4):
                    jc = 4 * s + jj
                    vp = ps_s.tile([P, C], f32, tag="ps_sp")
                    nc.tensor.matmul(
                        vp[:],
                        xs0[:, jj * P : (jj + 1) * P],
                        wv[:, 0, :],
                        start=True,
                        stop=False,
                    )
                    nc.tensor.matmul(
                        vp[:],
                        xs1[:, jj * P : (jj + 1) * P],
                        wv[:, 1, :],
                        start=False,
                        stop=True,
                    )
                    with nc.allow_low_precision(reason="f32r vt"):
                        if s < 4:
                            nc.scalar.copy(vt[:, jc, :], vp[:])
                        else:
                            nc.vector.tensor_copy(vt[:, jc, :], vp[:])

            # ---- attention blocks ----
            # den partial accumulators: dpA fed by DVE adds (eq rows 0,1 of
            # each quarter), dpB by GpSimd adds (rows 2,3); merged per block.
            dpA = s_pool.tile([P, NBLK, BLK], f32, tag="dpA")
            dpB = s_pool.tile([P, NBLK, BLK], f32, tag="dpB")

            hts = {}

            def oproj_tail(blk):
                h_t = hts.pop(blk)
                ib2 = blk * BLK
                for b in range(NCHUNK):
                    po = ps_misc.tile([P, BLK], f32, tag="ps_misc")
                    nc.tensor.matmul(
                        po[:], wo[:, 0, b, :], h_t[:, 0, :], start=True, stop=False
                    )
                    nc.tensor.matmul(
                        po[:], wo[:, 1, b, :], h_t[:, 1, :], start=False, stop=True
                    )
                    ot = o_pool.tile([P, BLK], f32, tag="o")
                    nc.vector.tensor_copy(ot[:], po[:])
                    nc.sync.dma_start(
                        out_d.ap().rearrange("a p n -> p a n")[:, b, ib2 : ib2 + BLK],
                        ot[:],
                    )

            def den_tail(blk):
                # merge partials, cross-partition ones-matmul, copy out
                dpm = scr_pool.tile([P, BLK], f32r, tag="dpm")
                with nc.allow_low_precision(reason="f32r for ones matmul"):
                    nc.vector.tensor_tensor(
                        dpm[:], dpA[:, blk, :], dpB[:, blk, :], op=OP.add
                    )
                den_ps = ps_misc.tile([1, BLK], f32, tag="ps_misc")
                nc.tensor.matmul(
                    den_ps[:], ones_c[:], dpm[:], start=True, stop=True
                )
                den_sb = o_pool.tile([1, BLK], f32, tag="den_sb")
                nc.scalar.copy(den_sb[:], den_ps[:])
                nc.sync.dma_start(den_d.ap()[:, blk * BLK : (blk + 1) * BLK], den_sb[:])

            NQ = NJC // QUART
            for blk in range(NBLK):
                ib = blk * BLK
                av = ps_av.tile([P, NCHUNK, BLK], f32, tag="ps_av")
                eqs = {}
                # software pipeline: scores/exp for quarter q are emitted one
                # step ahead of AV for quarter q-1, so PE always has score
                # matmuls to run while ACT computes the exp.
                for quart in range(NQ + 1):
                    if quart < NQ:
                        eq = big16_pool.tile([P, QUART, BLK], f32r, tag="big16")
                        eqs[quart] = eq
                        for pair in range(QUART // 2):
                            sp = ps_s.tile([P, 2, BLK], f32, tag="ps_sp")
                            for u in range(2):
                                jc = quart * QUART + pair * 2 + u
                                xj = xa if jc < 16 else xb
                                jo = (jc % 16) * P
                                nc.tensor.matmul(
                                    sp[:, u, :],
                                    xj[:, 0, jo : jo + P],
                                    r_t[:, 0, ib : ib + BLK],
                                    start=True,
                                    stop=False,
                                )
                                nc.tensor.matmul(
                                    sp[:, u, :],
                                    xj[:, 1, jo : jo + P],
                                    r_t[:, 1, ib : ib + BLK],
                                    start=False,
                                    stop=True,
                                )
                            nc.scalar.activation(
                                eq[:, 2 * pair : 2 * pair + 2, :],
                                sp[:],
                                AF.Exp,
                                bias=zb[:],
                                scale=SCALE,
                            )
                    if quart == 1 and blk > 0:
                        den_tail(blk - 1)
                    if quart == 2 and blk > 0:
                        oproj_tail(blk - 1)
                    if quart > 0:
                        q0 = quart - 1
                        eq = eqs.pop(q0)
                        for jj in range(QUART):
                            jc = q0 * QUART + jj
                            for m in range(NCHUNK):
                                nc.tensor.matmul(
                                    av[:, m, :],
                                    vt[:, jc, m * P : (m + 1) * P],
                                    eq[:, jj, :],
                                    start=(jc == 0),
                                    stop=(jc == NJC - 1),
                                )
                        # denominator partials (contiguous adds, DVE/GpSimd)
                        if q0 == 0:
                            nc.vector.tensor_tensor(
                                dpA[:, blk, :], eq[:, 0, :], eq[:, 1, :], op=OP.add
                            )
                            nc.gpsimd.tensor_tensor(
                                dpB[:, blk, :], eq[:, 2, :], eq[:, 3, :], op=OP.add
                            )
                        else:
                            t0 = scr_pool.tile([P, BLK], f32, tag="t0")
                            nc.vector.tensor_tensor(
                                t0[:], eq[:, 0, :], eq[:, 1, :], op=OP.add
                            )
                            nc.vector.tensor_tensor(
                                dpA[:, blk, :], dpA[:, blk, :], t0[:], op=OP.add
                            )
                            t1 = scr_pool.tile([P, BLK], f32, tag="t1")
                            nc.gpsimd.tensor_tensor(
                                t1[:], eq[:, 2, :], eq[:, 3, :], op=OP.add
                            )
                            nc.gpsimd.tensor_tensor(
                                dpB[:, blk, :], dpB[:, blk, :], t1[:], op=OP.add
                            )

                # h_unnorm psum -> sbuf (output projection deferred into the
                # next block's score stream)
                h_t = h_pool.tile([P, NCHUNK, BLK], f32r, tag="h")
                with nc.allow_low_precision(reason="f32r rounding for matmul feed"):
                    for m in range(NCHUNK):
                        nc.scalar.copy(h_t[:, m, :], av[:, m, :])
                hts[blk] = h_t

            oproj_tail(NBLK - 1)
            den_tail(NBLK - 1)

    nc.compile()
    return nc


def _prep_shards(x, gamma, beta, Wq, bq, Wk, bk, Wv, bv, Wo, bo):
    xr = np.ascontiguousarray(x, dtype=np.float32).reshape(4, C, N)
    gamma = np.asarray(gamma, np.float64)
    beta = np.asarray(beta, np.float64)
    Wq64 = np.asarray(Wq, np.float64)
    Wk64 = np.asarray(Wk, np.float64)
    Wv64 = np.asarray(Wv, np.float64)

    def w4(W):
        # w4[p, a, b, m] = W[b*128+m, a*128+p]
        return np.ascontiguousarray(
            np.asarray(W, np.float32).reshape(NCHUNK, P, NCHUNK, P).transpose(3, 2, 0, 1)
        )

    def wv3(W):
        return np.ascontiguousarray(
            np.asarray(W, np.float32).reshape(C, NCHUNK, P).transpose(2, 1, 0)
        )

    def b2(v):
        return np.ascontiguousarray(np.asarray(v, np.float32).reshape(NCHUNK, P).T)

    wo_h = w4(Wo)
    in_maps = []
    add_c = []
    for core in range(8):
        img = core // 2
        xi = xr[img].reshape(NCHUNK, P, N)
        if core % 2 == 0:
            xa_h, xb_h = xi[:, :, :NHALF], xi[:, :, NHALF:]
        else:
            xa_h, xb_h = xi[:, :, NHALF:], xi[:, :, :NHALF]
        if core % 2 == 0:
            # per-image GN affine folded into the projection weights/biases
            xg = xr[img].reshape(NG, GS * N).astype(np.float64)
            mean = xg.mean(axis=1)
            var = xg.var(axis=1)
            rstd = 1.0 / np.sqrt(var + EPS)
            scale_c = gamma * np.repeat(rstd, GS)  # [C]
            shift_c = beta - np.repeat(mean, GS) * scale_c  # [C]
            Wqp = Wq64 * scale_c[None, :]
            Wkp = Wk64 * scale_c[None, :]
            M = Wqp.T @ Wkp  # [c2, c']: r = M^T-contraction over x
            wq_f = w4(M.T)
            wv_f = wv3(Wv64 * scale_c[None, :])
            bq_f = b2(Wkp.T @ (np.asarray(bq, np.float64) + Wq64 @ shift_c))
            bvrow64 = np.asarray(bv, np.float64) + Wv64 @ shift_c
            add_c.append(np.asarray(Wo, np.float64) @ bvrow64 + np.asarray(bo, np.float64))
        m = {
            "wq": wq_f,
            "wv": wv_f,
            "wo": wo_h,
            "bq": bq_f,
            "xa": np.ascontiguousarray(xa_h),
            "xb": np.ascontiguousarray(xb_h),
        }
        in_maps.append(m)
    return in_maps, np.asarray(add_c, np.float64)


def kernel(x, gamma, beta, Wq, bq, Wk, bk, Wv, bv, Wo, bo, _trace=False):
    from concourse.bass_utils import run_bass_kernel_spmd

    if "nc" not in _CACHE:
        _CACHE["nc"] = _build_program()
    nc = _CACHE["nc"]

    in_maps, add_c = _prep_shards(x, gamma, beta, Wq, bq, Wk, bk, Wv, bv, Wo, bo)
    res = run_bass_kernel_spmd(nc, in_maps, core_ids=list(range(8)), trace=_trace)
    _CACHE["last_results"] = res

    x_np = np.ascontiguousarray(x, dtype=np.float32).reshape(4, C, N)
    y = np.empty((4, C, N), np.float32)
    for core in range(8):
        o = res.results[core]["out"].reshape(C, NHALF)
        den = res.results[core]["den"].reshape(1, NHALF)
        img = core // 2
        lo, hi = (0, NHALF) if core % 2 == 0 else (NHALF, N)
        y[img, :, lo:hi] = (
            x_np[img, :, lo:hi] + o / den + add_c[img].astype(np.float32)[:, None]
        )
    return y.reshape(4, C, 64, 64)



# revision 2
# speedup vs baseline: 1.0448x; 1.0448x over previous
"""AttnBlock (GroupNorm + 1-head spatial self-attention + residual) on 8 trn2 cores.

Sharding: B=4 images, 2 cores per image. Each core receives its full image
(K/V need all n=4096 positions) and computes the attention rows for its half
of the query positions. Odd cores receive the image rolled by 2048 along n so
every core runs the identical SPMD program.

All matmuls run as fp8e4m3 DoubleRow (2 MACs/PE-cell/cycle, 256-deep
contraction per pass): scores, AV, softmax denominator (ones-stationary
matmuls accumulating [1,512] in PSUM), output projection, and both small
projections. GroupNorm is folded into the projection weights on the host
(x feeds every matmul raw); softmax normalization commutes with the 1x1 conv
so the device returns O_unnorm (bf16) + den (f32) and the host applies
out = x + 4*O/den + add_c in fp32.

fp8 range management (e4m3 max 240): exp bias -3.5 (max logit ~7.6); host
scales wq=4*M / wv=16*Wv' / wo=16*Wo with compensating 1/4, 1/16, 1/64
scales on the PSUM->SBUF copies; den and the host-side 4x absorb the rest.
The GN bias-through-Wq term (~1e-2 on unit logits) is dropped — far below
fp8 noise (validated: rel err ~1e-2 vs the 2e-2 gate).

Startup: dummy bf16 matmuls warm the PE HAM clock gate (1.2 -> 2.4 GHz)
while inputs stream on the two hardware-DGE DMA queues (sync + scalar) in
2KB/partition lines; the vT projection is software-pipelined into block 0's
score/AV stream so the DVE-paced vt copies never gate the tensor engine.
"""

import numpy as np

N = 4096  # spatial positions per image
NHALF = 2048  # query positions per core
C = 256
P = 128
NCHUNK = 2
NG = 32  # groups
GS = 8  # channels per group
EPS = 1e-6
SCALE = float(C) ** -0.5  # 0.0625
EXPB = -3.5  # exp bias: keeps e' = exp(s*SCALE+EXPB) inside fp8 range
NBLK = 4  # i-blocks of 512 per core
BLK = 512
NJC = 32  # j-chunks of 128
QUART = 4  # j-chunks per exp quarter-buffer
NWARM = 20  # HAM warmup matmuls (~4.3us of PE busy)

_CACHE = {}


def _build_program():
    import concourse.bacc as bacc
    import concourse.mybir as mybir
    import concourse.tile as tile

    f32 = mybir.dt.float32
    bf16 = mybir.dt.bfloat16
    f8 = mybir.dt.float8e4
    u8 = mybir.dt.uint8
    AF = mybir.ActivationFunctionType
    DR = mybir.MatmulPerfMode.DoubleRow

    nc = bacc.Bacc("TRN2", target_bir_lowering=False)

    # DRAM I/O. x8 is strip-major [P, strip, chunk, 1024] so each strip DMA
    # moves a contiguous 2KB line per partition.
    x8_d = nc.dram_tensor("x8", [P, 4, NCHUNK, 1024], f8, kind="ExternalInput")
    wq8_d = nc.dram_tensor("wq8", [P, NCHUNK, NCHUNK, P], f8, kind="ExternalInput")
    wv8_d = nc.dram_tensor("wv8", [P, NCHUNK, C], f8, kind="ExternalInput")
    wo8_d = nc.dram_tensor("wo8", [P, NCHUNK, NCHUNK, P], f8, kind="ExternalInput")
    out_d = nc.dram_tensor("out", [NCHUNK, P, NHALF], bf16, kind="ExternalOutput")
    den_d = nc.dram_tensor("den", [1, NHALF], f32, kind="ExternalOutput")

    def xj(x8t, jc):
        """lhsT pair [128, 2, 128] for j-chunk jc (columns jc*128..+128)."""
        return x8t[:, jc // 8, :, (jc % 8) * P : (jc % 8) * P + P]

    def xi(x8t, s):
        """rhs pair [128, 2, 512] for i-strip s (columns s*512..+512)."""
        return x8t[:, s // 2, :, (s % 2) * BLK : (s % 2) * BLK + BLK]

    with tile.TileContext(nc) as tc:
        with (
            tc.tile_pool(name="warm", bufs=1) as warm_pool,
            tc.tile_pool(name="xpool", bufs=1) as x_pool,
            tc.tile_pool(name="wpool", bufs=1) as w_pool,
            tc.tile_pool(name="rpool", bufs=1) as r_pool,
            tc.tile_pool(name="vpool", bufs=1) as v_pool,
            tc.tile_pool(name="eq", bufs=3) as eq_pool,
            tc.tile_pool(name="hpool", bufs=2) as h_pool,
            tc.tile_pool(name="opool", bufs=3) as o_pool,
            tc.tile_pool(name="small", bufs=1) as s_pool,
            tc.tile_pool(name="ps_s", bufs=2, space="PSUM") as ps_s,
            tc.tile_pool(name="ps_av", bufs=1, space="PSUM") as ps_av,
            tc.tile_pool(name="ps_den", bufs=1, space="PSUM") as ps_den,
            tc.tile_pool(name="ps_vp", bufs=1, space="PSUM") as ps_vp,
        ):
            # ---- constants (gpsimd queue: memsets only, so they run first)
            wtile = warm_pool.tile([P, BLK], bf16, tag="warm")
            nc.gpsimd.memset(wtile[:].bitcast(mybir.dt.uint16), 0)
            eb = s_pool.tile([P, 1], f32, tag="eb")
            nc.gpsimd.memset(eb[:], EXPB)
            ones8 = s_pool.tile([P, NCHUNK, 16], f8, tag="ones8")
            nc.gpsimd.memset(ones8[:].bitcast(u8), 0x38)  # fp8e4m3 1.0

            # ---- PE warmup: trip the HAM clock gate while DMAs stream ----
            for _ in range(NWARM):
                wps = ps_s.tile([P, NCHUNK, BLK], f32, tag="sp")
                nc.tensor.matmul(
                    wps[:, 0, :], wtile[:, 0:P], wtile[:], start=True, stop=True
                )

            # ---- input loads: 2 HW-DGE queues, first-needed first ----
            wq8 = w_pool.tile([P, NCHUNK, NCHUNK, P], f8, tag="wq8")
            nc.sync.dma_start(wq8[:], wq8_d.ap())
            wv8 = w_pool.tile([P, NCHUNK, C], f8, tag="wv8")
            nc.scalar.dma_start(wv8[:], wv8_d.ap())
            x8 = x_pool.tile([P, 4, NCHUNK, 1024], f8, tag="x8")
            for s in range(2):
                nc.sync.dma_start(x8[:, s, :, :], x8_d.ap()[:, s, :, :])
            for s in range(2, 4):
                nc.scalar.dma_start(x8[:, s, :, :], x8_d.ap()[:, s, :, :])
            wo8 = w_pool.tile([P, NCHUNK, NCHUNK, P], f8, tag="wo8")
            nc.scalar.dma_start(wo8[:], wo8_d.ap())

            r8 = r_pool.tile([P, NCHUNK, NHALF], f8, tag="r8")
            vt8 = v_pool.tile([P, NJC, C], f8, tag="vt8")

            # ---- r projection (8 DR matmuls) + vt pairs 0-3 upfront ----
            def emit_r_strip(s, split=False):
                rp = ps_s.tile([P, NCHUNK, BLK], f32, tag="sp")
                for b in range(NCHUNK):
                    nc.tensor.matmul(
                        rp[:, b, :],
                        wq8[:, :, b, :],
                        xi(x8, s),
                        start=True,
                        stop=True,
                        perf_mode=DR,
                    )
                sl = slice(s * BLK, (s + 1) * BLK)
                with nc.allow_low_precision(reason="fp8 r"):
                    if split:
                        nc.vector.tensor_scalar_mul(r8[:, 0, sl], rp[:, 0, :], 0.25)
                        nc.scalar.activation(r8[:, 1, sl], rp[:, 1, :], AF.Copy, scale=0.25)
                    else:
                        nc.vector.tensor_scalar_mul(r8[:, :, sl], rp[:], 0.25)

            def emit_vt_pair_mm(pair):
                vp = ps_vp.tile([P, NCHUNK, C], f32, tag="vp")
                for jj in range(2):
                    jc = 2 * pair + jj
                    nc.tensor.matmul(
                        vp[:, jj, :],
                        xj(x8, jc),
                        wv8[:],
                        start=True,
                        stop=True,
                        perf_mode=DR,
                    )
                return vp

            def emit_vt_pair_copy(pair, vp, eng="dve"):
                with nc.allow_low_precision(reason="fp8 vt"):
                    if eng == "act":
                        nc.scalar.activation(
                            vt8[:, 2 * pair : 2 * pair + 2, :],
                            vp[:],
                            AF.Copy,
                            scale=1 / 16.0,
                        )
                    else:
                        nc.vector.tensor_scalar_mul(
                            vt8[:, 2 * pair : 2 * pair + 2, :], vp[:], 1 / 16.0
                        )

            emit_r_strip(0, split=True)

            # ---- attention blocks ----
            hts = {}
            dens = {}

            def oproj_tail(blk, fast=False):
                h8 = hts.pop(blk)
                ib2 = blk * BLK
                sl = slice(ib2, ib2 + BLK)
                ob = o_pool.tile([P, NCHUNK, BLK], bf16, tag="ob")
                for b in range(NCHUNK):
                    po = ps_s.tile([P, NCHUNK, BLK], f32, tag="sp")
                    nc.tensor.matmul(
                        po[:, 0, :],
                        wo8[:, :, b, :],
                        h8[:],
                        start=True,
                        stop=True,
                        perf_mode=DR,
                    )
                    with nc.allow_low_precision(reason="bf16 out"):
                        if fast and b == 1:
                            nc.scalar.activation(ob[:, b, :], po[:, 0, :], AF.Copy)
                        else:
                            nc.vector.tensor_copy(ob[:, b, :], po[:, 0, :])
                    if fast:
                        eng = nc.scalar if b == 1 else nc.sync
                        eng.dma_start(out_d.ap()[b, :, sl], ob[:, b, :])
                if not fast:
                    nc.sync.dma_start(
                        out_d.ap().rearrange("a p n -> p a n")[:, :, sl], ob[:]
                    )

            den_sb = s_pool.tile([1, NHALF], f32, tag="den_sb")

            def den_tail(blk):
                denp = dens.pop(blk)
                nc.vector.tensor_copy(
                    den_sb[:, blk * BLK : (blk + 1) * BLK], denp[:]
                )
                if blk == NBLK - 1:
                    nc.scalar.dma_start(den_d.ap(), den_sb[:])

            NQ = NJC // QUART
            for blk in range(NBLK):
                ib = blk * BLK
                av = ps_av.tile([P, NCHUNK, BLK], f32, tag="av")
                denp = ps_den.tile([1, BLK], f32, tag="den")
                dens[blk] = denp
                eqs = {}
                # software pipeline: scores/exp for quarter q one step ahead
                # of AV/den for quarter q-1. During block 0 the remaining vT
                # projection pairs (4-15) are drizzled in two per quarter.
                for quart in range(NQ + 1):
                    if quart < NQ:
                        eq = eq_pool.tile([P, QUART, BLK], f8, tag="eq")
                        eqs[quart] = eq
                        for u in range(2):
                            sp = ps_s.tile([P, 2, BLK], f32, tag="sp")
                            for t in range(2):
                                jc = QUART * quart + 2 * u + t
                                nc.tensor.matmul(
                                    sp[:, t, :],
                                    xj(x8, jc),
                                    r8[:, :, ib : ib + BLK],
                                    start=True,
                                    stop=True,
                                    perf_mode=DR,
                                )
                            with nc.allow_low_precision(reason="fp8 exp"):
                                nc.scalar.activation(
                                    eq[:, 2 * u : 2 * u + 2, :],
                                    sp[:],
                                    AF.Exp,
                                    bias=eb[:],
                                    scale=SCALE,
                                )
                    # block 0 streams the vT projection: pairs 0-3 burst in
                    # quarter 0 (copies alternate DVE/ACT while ACT idles
                    # during pipeline priming), pairs (2q+2, 2q+3) inside
                    # quarter q afterwards; AV needs a pair a quarter later.
                    if blk == 0 and quart == 0:
                        for pair in range(4):
                            vp0 = emit_vt_pair_mm(pair)
                            emit_vt_pair_copy(pair, vp0, "act" if pair % 2 else "dve")
                    elif blk == 0 and 1 <= quart <= 6:
                        vp0 = emit_vt_pair_mm(2 * quart + 2)
                        emit_vt_pair_copy(2 * quart + 2, vp0)
                    if quart == 2 and blk > 0:
                        oproj_tail(blk - 1)
                    if quart == 5 and blk < NBLK - 1:
                        emit_r_strip(blk + 1)
                    if quart > 0:
                        q0 = quart - 1
                        eq = eqs.pop(q0)
                        for u in range(2):
                            pr = 2 * q0 + u  # pair index 0..15
                            jc0 = QUART * q0 + 2 * u
                            for m in range(NCHUNK):
                                nc.tensor.matmul(
                                    av[:, m, :],
                                    vt8[:, jc0 : jc0 + 2, m * P : (m + 1) * P],
                                    eq[:, 2 * u : 2 * u + 2, :],
                                    start=(pr == 0),
                                    stop=(pr == 15),
                                    perf_mode=DR,
                                )
                            nc.tensor.matmul(
                                denp[:],
                                ones8[:, :, 0:1],
                                eq[:, 2 * u : 2 * u + 2, :],
                                start=(pr == 0),
                                stop=(pr == 15),
                                perf_mode=DR,
                            )
                            if u == 0 and blk == 0 and 1 <= quart <= 6:
                                vp1 = emit_vt_pair_mm(2 * quart + 3)
                                emit_vt_pair_copy(2 * quart + 3, vp1)


                den_tail(blk)
                # h8 = av/64 (fp8, DVE), read by next block's oproj_tail.
                # Final block: split across DVE+ACT so the tail drains fast.
                h8 = h_pool.tile([P, NCHUNK, BLK], f8, tag="h8")
                with nc.allow_low_precision(reason="fp8 h"):
                    if blk == NBLK - 1:
                        nc.vector.tensor_scalar_mul(h8[:, 0, :], av[:, 0, :], 1 / 64.0)
                        nc.scalar.activation(
                            h8[:, 1, :], av[:, 1, :], AF.Copy, scale=1 / 64.0
                        )
                    else:
                        nc.vector.tensor_scalar_mul(h8[:], av[:], 1 / 64.0)
                hts[blk] = h8

            oproj_tail(NBLK - 1, fast=True)

    nc.compile()
    return nc


def _prep_shards(x, gamma, beta, Wq, bq, Wk, bk, Wv, bv, Wo, bo):
    import ml_dtypes

    E4 = ml_dtypes.float8_e4m3

    xr = np.ascontiguousarray(x, dtype=np.float32).reshape(4, C, N)
    gamma = np.asarray(gamma, np.float64)
    beta = np.asarray(beta, np.float64)
    Wq64 = np.asarray(Wq, np.float64)
    Wk64 = np.asarray(Wk, np.float64)
    Wv64 = np.asarray(Wv, np.float64)
    Wo64 = np.asarray(Wo, np.float64)

    def w4(W):
        # w4[p, a, b, m] = W[b*128+m, a*128+p]
        return np.ascontiguousarray(
            np.asarray(W, np.float32)
            .reshape(NCHUNK, P, NCHUNK, P)
            .transpose(3, 2, 0, 1)
            .astype(E4)
        )

    def wv3(W):
        return np.ascontiguousarray(
            np.asarray(W, np.float32).reshape(C, NCHUNK, P).transpose(2, 1, 0).astype(E4)
        )

    in_maps = []
    add_c = []
    per_img = {}
    for core in range(8):
        img = core // 2
        if core % 2 == 0:
            xi = xr[img]  # [C, N]
            xg = xi.reshape(NG, GS * N).astype(np.float64)
            mean = xg.mean(axis=1)
            var = xg.var(axis=1)
            rstd = 1.0 / np.sqrt(var + EPS)
            scale_c = gamma * np.repeat(rstd, GS)
            shift_c = beta - np.repeat(mean, GS) * scale_c
            Wqp = Wq64 * scale_c[None, :]
            Wkp = Wk64 * scale_c[None, :]
            M = Wqp.T @ Wkp
            bvrow = np.asarray(bv, np.float64) + Wv64 @ shift_c
            add_c.append(Wo64 @ bvrow + np.asarray(bo, np.float64))
            xc = xi.reshape(NCHUNK, P, N).transpose(1, 0, 2)  # [P, 2, N]
            per_img = {
                "wq8": w4(4.0 * M.T),
                "wv8": wv3(16.0 * Wv64 * scale_c[None, :]),
                "wo8": w4(16.0 * Wo64),
                "x": np.ascontiguousarray(xc),
            }
        xc = per_img["x"]
        if core % 2 == 1:
            xc = np.roll(xc, -NHALF, axis=2)
        # strip-major fp8: [P, strip, chunk, 1024]
        x8 = np.ascontiguousarray(
            xc.reshape(P, NCHUNK, 4, 1024).transpose(0, 2, 1, 3).astype(E4)
        )
        m = {k: v for k, v in per_img.items() if k != "x"}
        m["x8"] = x8
        in_maps.append(m)
    return in_maps, np.asarray(add_c, np.float64)


def kernel(x, gamma, beta, Wq, bq, Wk, bk, Wv, bv, Wo, bo, _trace=False):
    from concourse.bass_utils import run_bass_kernel_spmd

    if "nc" not in _CACHE:
        _CACHE["nc"] = _build_program()
    nc = _CACHE["nc"]

    in_maps, add_c = _prep_shards(x, gamma, beta, Wq, bq, Wk, bk, Wv, bv, Wo, bo)
    res = run_bass_kernel_spmd(nc, in_maps, core_ids=list(range(8)), trace=_trace)
    _CACHE["last_results"] = res

    x_np = np.ascontiguousarray(x, dtype=np.float32).reshape(4, C, N)
    y = np.empty((4, C, N), np.float32)
    for core in range(8):
        o = res.results[core]["out"].astype(np.float32).reshape(C, NHALF)
        den = res.results[core]["den"].astype(np.float32).reshape(1, NHALF)
        img = core // 2
        lo, hi = (0, NHALF) if core % 2 == 0 else (NHALF, N)
        y[img, :, lo:hi] = (
            x_np[img, :, lo:hi] + 4.0 * o / den + add_c[img].astype(np.float32)[:, None]
        )
    return y.reshape(4, C, 64, 64)


# revision 3
# speedup vs baseline: 1.0503x; 1.0053x over previous
"""AttnBlock (GroupNorm + 1-head spatial self-attention + residual) on 8 trn2 cores.

Sharding: B=4 images, 2 cores per image. Each core receives its full image
(K/V need all n=4096 positions) and computes the attention rows for its half
of the query positions. Odd cores receive the image rolled by 2048 along n so
every core runs the identical SPMD program.

All matmuls run as fp8e4m3 DoubleRow (2 MACs/PE-cell/cycle, 256-deep
contraction per pass): scores, AV, softmax denominator (ones-stationary
matmuls accumulating [1,512] in PSUM), output projection, and both small
projections. GroupNorm is folded into the projection weights on the host
(x feeds every matmul raw); softmax normalization commutes with the 1x1 conv
so the device returns O_unnorm (bf16) + den (f32) and the host applies
out = x + 4*O/den + add_c in fp32.

fp8 range management (e4m3 max 240): exp bias -3.5 (max logit ~7.6); host
scales wq=4*M / wv=16*Wv' / wo=16*Wo with compensating 1/4, 1/16, 1/64
scales on the PSUM->SBUF copies; den and the host-side 4x absorb the rest.
The GN bias-through-Wq term (~1e-2 on unit logits) is dropped — far below
fp8 noise (validated: rel err ~1e-2 vs the 2e-2 gate).

Startup: dummy bf16 matmuls warm the PE HAM clock gate (1.2 -> 2.4 GHz)
while inputs stream on the two hardware-DGE DMA queues (sync + scalar) in
2KB/partition lines; the vT projection is software-pipelined into block 0's
score/AV stream so the DVE-paced vt copies never gate the tensor engine.
"""

import numpy as np

N = 4096  # spatial positions per image
NHALF = 2048  # query positions per core
C = 256
P = 128
NCHUNK = 2
NG = 32  # groups
GS = 8  # channels per group
EPS = 1e-6
SCALE = float(C) ** -0.5  # 0.0625
EXPB = -3.5  # exp bias: keeps e' = exp(s*SCALE+EXPB) inside fp8 range
NBLK = 4  # i-blocks of 512 per core
BLK = 512
NJC = 32  # j-chunks of 128
QUART = 4  # j-chunks per exp quarter-buffer
NWARM = 20  # HAM warmup matmuls (~4.3us of PE busy)

_CACHE = {}


def _build_program():
    import concourse.bacc as bacc
    import concourse.mybir as mybir
    import concourse.tile as tile

    f32 = mybir.dt.float32
    bf16 = mybir.dt.bfloat16
    f8 = mybir.dt.float8e4
    u8 = mybir.dt.uint8
    AF = mybir.ActivationFunctionType
    DR = mybir.MatmulPerfMode.DoubleRow

    nc = bacc.Bacc("TRN2", target_bir_lowering=False)

    # DRAM I/O. x8 is strip-major [P, strip, chunk, 1024] so each strip DMA
    # moves a contiguous 2KB line per partition.
    x8_d = nc.dram_tensor("x8", [P, 4, NCHUNK, 1024], f8, kind="ExternalInput")
    wq8_d = nc.dram_tensor("wq8", [P, NCHUNK, NCHUNK, P], f8, kind="ExternalInput")
    wu8_d = nc.dram_tensor("wu8", [P, NCHUNK, C], f8, kind="ExternalInput")
    out_d = nc.dram_tensor("out", [NCHUNK, P, NHALF], bf16, kind="ExternalOutput")
    den_d = nc.dram_tensor("den", [1, NHALF], f32, kind="ExternalOutput")

    def xj(x8t, jc):
        """lhsT pair [128, 2, 128] for j-chunk jc (columns jc*128..+128)."""
        return x8t[:, jc // 8, :, (jc % 8) * P : (jc % 8) * P + P]

    def xi(x8t, s):
        """rhs pair [128, 2, 512] for i-strip s (columns s*512..+512)."""
        return x8t[:, s // 2, :, (s % 2) * BLK : (s % 2) * BLK + BLK]

    with tile.TileContext(nc) as tc:
        with (
            tc.tile_pool(name="warm", bufs=1) as warm_pool,
            tc.tile_pool(name="xpool", bufs=1) as x_pool,
            tc.tile_pool(name="wpool", bufs=1) as w_pool,
            tc.tile_pool(name="rpool", bufs=1) as r_pool,
            tc.tile_pool(name="vpool", bufs=1) as v_pool,
            tc.tile_pool(name="eq", bufs=3) as eq_pool,
            tc.tile_pool(name="hpool", bufs=2) as h_pool,
            tc.tile_pool(name="opool", bufs=3) as o_pool,
            tc.tile_pool(name="small", bufs=1) as s_pool,
            tc.tile_pool(name="ps_s", bufs=2, space="PSUM") as ps_s,
            tc.tile_pool(name="ps_av", bufs=1, space="PSUM") as ps_av,
            tc.tile_pool(name="ps_den", bufs=1, space="PSUM") as ps_den,
            tc.tile_pool(name="ps_vp", bufs=1, space="PSUM") as ps_vp,
        ):
            # ---- constants (gpsimd queue: memsets only, so they run first)
            wtile = warm_pool.tile([P, BLK], bf16, tag="warm")
            nc.gpsimd.memset(wtile[:].bitcast(mybir.dt.uint16), 0)
            eb = s_pool.tile([P, 1], f32, tag="eb")
            nc.gpsimd.memset(eb[:], EXPB)
            ones8 = s_pool.tile([P, NCHUNK, 16], f8, tag="ones8")
            nc.gpsimd.memset(ones8[:].bitcast(u8), 0x38)  # fp8e4m3 1.0

            # ---- PE warmup: trip the HAM clock gate while DMAs stream ----
            for _ in range(NWARM):
                wps = ps_s.tile([P, NCHUNK, BLK], f32, tag="sp")
                nc.tensor.matmul(
                    wps[:, 0, :], wtile[:, 0:P], wtile[:], start=True, stop=True
                )

            # ---- input loads: 2 HW-DGE queues, first-needed first ----
            wq8 = w_pool.tile([P, NCHUNK, NCHUNK, P], f8, tag="wq8")
            nc.sync.dma_start(wq8[:], wq8_d.ap())
            wu8 = w_pool.tile([P, NCHUNK, C], f8, tag="wu8")
            nc.scalar.dma_start(wu8[:], wu8_d.ap())
            x8 = x_pool.tile([P, 4, NCHUNK, 1024], f8, tag="x8")
            for s in range(2):
                nc.sync.dma_start(x8[:, s, :, :], x8_d.ap()[:, s, :, :])
            for s in range(2, 4):
                nc.scalar.dma_start(x8[:, s, :, :], x8_d.ap()[:, s, :, :])

            r8 = r_pool.tile([P, NCHUNK, NHALF], f8, tag="r8")
            vt8 = v_pool.tile([P, NJC, C], f8, tag="vt8")

            # ---- r projection (8 DR matmuls) + vt pairs 0-3 upfront ----
            def emit_r_strip(s, split=False):
                rp = ps_s.tile([P, NCHUNK, BLK], f32, tag="sp")
                for b in range(NCHUNK):
                    nc.tensor.matmul(
                        rp[:, b, :],
                        wq8[:, :, b, :],
                        xi(x8, s),
                        start=True,
                        stop=True,
                        perf_mode=DR,
                    )
                sl = slice(s * BLK, (s + 1) * BLK)
                with nc.allow_low_precision(reason="fp8 r"):
                    if split:
                        nc.vector.tensor_scalar_mul(r8[:, 0, sl], rp[:, 0, :], 0.25)
                        nc.scalar.activation(r8[:, 1, sl], rp[:, 1, :], AF.Copy, scale=0.25)
                    else:
                        nc.vector.tensor_scalar_mul(r8[:, :, sl], rp[:], 0.25)

            def emit_vt_pair_mm(pair):
                vp = ps_vp.tile([P, NCHUNK, C], f32, tag="vp")
                for jj in range(2):
                    jc = 2 * pair + jj
                    nc.tensor.matmul(
                        vp[:, jj, :],
                        xj(x8, jc),
                        wu8[:],
                        start=True,
                        stop=True,
                        perf_mode=DR,
                    )
                return vp

            def emit_vt_pair_copy(pair, vp, eng="dve"):
                with nc.allow_low_precision(reason="fp8 vt"):
                    if eng == "act":
                        nc.scalar.activation(
                            vt8[:, 2 * pair : 2 * pair + 2, :],
                            vp[:],
                            AF.Copy,
                            scale=1 / 16.0,
                        )
                    else:
                        nc.vector.tensor_scalar_mul(
                            vt8[:, 2 * pair : 2 * pair + 2, :], vp[:], 1 / 16.0
                        )

            emit_r_strip(0, split=True)

            # ---- attention blocks ----
            avs = {}
            dens = {}

            def out_tail(blk, fast=False):
                # Wo is folded into the AV weights (Wu = Wo @ Wv'), so the av
                # accumulator IS the projected output: just copy + DMA.
                avb = avs.pop(blk)
                sl = slice(blk * BLK, (blk + 1) * BLK)
                ob = o_pool.tile([P, NCHUNK, BLK], bf16, tag="ob")
                with nc.allow_low_precision(reason="bf16 out"):
                    if fast:
                        nc.vector.tensor_copy(ob[:, 0, :], avb[:, 0, :])
                        nc.scalar.activation(ob[:, 1, :], avb[:, 1, :], AF.Copy)
                        nc.sync.dma_start(out_d.ap()[0, :, sl], ob[:, 0, :])
                        nc.scalar.dma_start(out_d.ap()[1, :, sl], ob[:, 1, :])
                    else:
                        nc.vector.tensor_copy(ob[:], avb[:])
                        nc.sync.dma_start(
                            out_d.ap().rearrange("a p n -> p a n")[:, :, sl], ob[:]
                        )

            den_sb = s_pool.tile([1, NHALF], f32, tag="den_sb")

            def den_tail(blk):
                denp = dens.pop(blk)
                nc.vector.tensor_copy(
                    den_sb[:, blk * BLK : (blk + 1) * BLK], denp[:]
                )
                if blk == NBLK - 1:
                    nc.scalar.dma_start(den_d.ap(), den_sb[:])

            NQ = NJC // QUART
            for blk in range(NBLK):
                ib = blk * BLK
                av = ps_av.tile([P, NCHUNK, BLK], f32, tag="av")
                denp = ps_den.tile([1, BLK], f32, tag="den")
                dens[blk] = denp
                eqs = {}
                # software pipeline: scores/exp for quarter q one step ahead
                # of AV/den for quarter q-1. During block 0 the remaining vT
                # projection pairs (4-15) are drizzled in two per quarter.
                for quart in range(NQ + 1):
                    if quart < NQ:
                        eq = eq_pool.tile([P, QUART, BLK], f8, tag="eq")
                        eqs[quart] = eq
                        for u in range(2):
                            sp = ps_s.tile([P, 2, BLK], f32, tag="sp")
                            for t in range(2):
                                jc = QUART * quart + 2 * u + t
                                nc.tensor.matmul(
                                    sp[:, t, :],
                                    xj(x8, jc),
                                    r8[:, :, ib : ib + BLK],
                                    start=True,
                                    stop=True,
                                    perf_mode=DR,
                                )
                            with nc.allow_low_precision(reason="fp8 exp"):
                                nc.scalar.activation(
                                    eq[:, 2 * u : 2 * u + 2, :],
                                    sp[:],
                                    AF.Exp,
                                    bias=eb[:],
                                    scale=SCALE,
                                )
                    # block 0 streams the vT projection: pairs 0-3 burst in
                    # quarter 0 (copies alternate DVE/ACT while ACT idles
                    # during pipeline priming), pairs (2q+2, 2q+3) inside
                    # quarter q afterwards; AV needs a pair a quarter later.
                    if blk == 0 and quart == 0:
                        for pair in range(4):
                            vp0 = emit_vt_pair_mm(pair)
                            emit_vt_pair_copy(pair, vp0, "act" if pair % 2 else "dve")
                    elif blk == 0 and 1 <= quart <= 6:
                        vp0 = emit_vt_pair_mm(2 * quart + 2)
                        emit_vt_pair_copy(2 * quart + 2, vp0)
                    if quart == 5 and blk < NBLK - 1:
                        emit_r_strip(blk + 1)
                    if quart > 0:
                        q0 = quart - 1
                        eq = eqs.pop(q0)
                        for u in range(2):
                            pr = 2 * q0 + u  # pair index 0..15
                            jc0 = QUART * q0 + 2 * u
                            for m in range(NCHUNK):
                                nc.tensor.matmul(
                                    av[:, m, :],
                                    vt8[:, jc0 : jc0 + 2, m * P : (m + 1) * P],
                                    eq[:, 2 * u : 2 * u + 2, :],
                                    start=(pr == 0),
                                    stop=(pr == 15),
                                    perf_mode=DR,
                                )
                            nc.tensor.matmul(
                                denp[:],
                                ones8[:, :, 0:1],
                                eq[:, 2 * u : 2 * u + 2, :],
                                start=(pr == 0),
                                stop=(pr == 15),
                                perf_mode=DR,
                            )
                            if u == 0 and blk == 0 and 1 <= quart <= 6:
                                vp1 = emit_vt_pair_mm(2 * quart + 3)
                                emit_vt_pair_copy(2 * quart + 3, vp1)


                den_tail(blk)
                avs[blk] = av
                out_tail(blk, fast=(blk == NBLK - 1))

    nc.compile()
    return nc


def _prep_shards(x, gamma, beta, Wq, bq, Wk, bk, Wv, bv, Wo, bo):
    import ml_dtypes

    E4 = ml_dtypes.float8_e4m3

    xr = np.ascontiguousarray(x, dtype=np.float32).reshape(4, C, N)
    gamma = np.asarray(gamma, np.float64)
    beta = np.asarray(beta, np.float64)
    Wq64 = np.asarray(Wq, np.float64)
    Wk64 = np.asarray(Wk, np.float64)
    Wv64 = np.asarray(Wv, np.float64)
    Wo64 = np.asarray(Wo, np.float64)

    def w4(W):
        # w4[p, a, b, m] = W[b*128+m, a*128+p]
        return np.ascontiguousarray(
            np.asarray(W, np.float32)
            .reshape(NCHUNK, P, NCHUNK, P)
            .transpose(3, 2, 0, 1)
            .astype(E4)
        )

    def wv3(W):
        return np.ascontiguousarray(
            np.asarray(W, np.float32).reshape(C, NCHUNK, P).transpose(2, 1, 0).astype(E4)
        )

    in_maps = []
    add_c = []
    per_img = {}
    for core in range(8):
        img = core // 2
        if core % 2 == 0:
            xi = xr[img]  # [C, N]
            xg = xi.reshape(NG, GS * N).astype(np.float64)
            mean = xg.mean(axis=1)
            var = xg.var(axis=1)
            rstd = 1.0 / np.sqrt(var + EPS)
            scale_c = gamma * np.repeat(rstd, GS)
            shift_c = beta - np.repeat(mean, GS) * scale_c
            Wqp = Wq64 * scale_c[None, :]
            Wkp = Wk64 * scale_c[None, :]
            M = Wqp.T @ Wkp
            bvrow = np.asarray(bv, np.float64) + Wv64 @ shift_c
            add_c.append(Wo64 @ bvrow + np.asarray(bo, np.float64))
            xc = xi.reshape(NCHUNK, P, N).transpose(1, 0, 2)  # [P, 2, N]
            Wu = Wo64 @ (Wv64 * scale_c[None, :])
            per_img = {
                "wq8": w4(4.0 * M.T),
                "wu8": wv3(16.0 * Wu),
                "x": np.ascontiguousarray(xc),
            }
        xc = per_img["x"]
        if core % 2 == 1:
            xc = np.roll(xc, -NHALF, axis=2)
        # strip-major fp8: [P, strip, chunk, 1024]
        x8 = np.ascontiguousarray(
            xc.reshape(P, NCHUNK, 4, 1024).transpose(0, 2, 1, 3).astype(E4)
        )
        m = {k: v for k, v in per_img.items() if k != "x"}
        m["x8"] = x8
        in_maps.append(m)
    return in_maps, np.asarray(add_c, np.float64)


def kernel(x, gamma, beta, Wq, bq, Wk, bk, Wv, bv, Wo, bo, _trace=False):
    from concourse.bass_utils import run_bass_kernel_spmd

    if "nc" not in _CACHE:
        _CACHE["nc"] = _build_program()
    nc = _CACHE["nc"]

    in_maps, add_c = _prep_shards(x, gamma, beta, Wq, bq, Wk, bk, Wv, bv, Wo, bo)
    res = run_bass_kernel_spmd(nc, in_maps, core_ids=list(range(8)), trace=_trace)
    _CACHE["last_results"] = res

    x_np = np.ascontiguousarray(x, dtype=np.float32).reshape(4, C, N)
    y = np.empty((4, C, N), np.float32)
    for core in range(8):
        o = res.results[core]["out"].astype(np.float32).reshape(C, NHALF)
        den = res.results[core]["den"].astype(np.float32).reshape(1, NHALF)
        img = core // 2
        lo, hi = (0, NHALF) if core % 2 == 0 else (NHALF, N)
        y[img, :, lo:hi] = (
            x_np[img, :, lo:hi] + o / den + add_c[img].astype(np.float32)[:, None]
        )
    return y.reshape(4, C, 64, 64)


# revision 4
# speedup vs baseline: 1.0654x; 1.0144x over previous
"""AttnBlock (GroupNorm + 1-head spatial self-attention + residual) on 8 trn2 cores.

Sharding: B=4 images, 2 cores per image. Each core receives its full image
(K/V need all n=4096 positions) and computes the attention rows for its half
of the query positions. Odd cores receive the image rolled by 2048 along n so
every core runs the identical SPMD program.

All matmuls run as fp8e4m3 DoubleRow (2 MACs/PE-cell/cycle, 256-deep
contraction per pass): scores, AV, softmax denominator (ones-stationary
matmuls accumulating [1,512] in PSUM), output projection, and both small
projections. GroupNorm is folded into the projection weights on the host
(x feeds every matmul raw); softmax normalization commutes with the 1x1 conv
so the device returns O_unnorm (bf16) + den (f32) and the host applies
out = x + 4*O/den + add_c in fp32.

fp8 range management (e4m3 max 240): exp bias -3.5 (max logit ~7.6); host
scales wq=4*M / wv=16*Wv' / wo=16*Wo with compensating 1/4, 1/16, 1/64
scales on the PSUM->SBUF copies; den and the host-side 4x absorb the rest.
The GN bias-through-Wq term (~1e-2 on unit logits) is dropped — far below
fp8 noise (validated: rel err ~1e-2 vs the 2e-2 gate).

Startup: dummy bf16 matmuls warm the PE HAM clock gate (1.2 -> 2.4 GHz)
while inputs stream on the two hardware-DGE DMA queues (sync + scalar) in
2KB/partition lines; the vT projection is software-pipelined into block 0's
score/AV stream so the DVE-paced vt copies never gate the tensor engine.
"""

import numpy as np

N = 4096  # spatial positions per image
NHALF = 2048  # query positions per core
C = 256
P = 128
NCHUNK = 2
NG = 32  # groups
GS = 8  # channels per group
EPS = 1e-6
SCALE = float(C) ** -0.5  # 0.0625
EXPB = -3.5  # exp bias: keeps e' = exp(s*SCALE+EXPB) inside fp8 range
NBLK = 4  # i-blocks of 512 per core
BLK = 512
NJC = 32  # j-chunks of 128
QUART = 4  # j-chunks per exp quarter-buffer
NWARM = 20  # HAM warmup matmuls (~4.3us of PE busy)

_CACHE = {}


def _build_program():
    import concourse.bacc as bacc
    import concourse.mybir as mybir
    import concourse.tile as tile

    f32 = mybir.dt.float32
    bf16 = mybir.dt.bfloat16
    f8 = mybir.dt.float8e4
    u8 = mybir.dt.uint8
    AF = mybir.ActivationFunctionType
    DR = mybir.MatmulPerfMode.DoubleRow

    nc = bacc.Bacc("TRN2", target_bir_lowering=False)

    # DRAM I/O. x8 is strip-major [P, strip, chunk, 1024] so each strip DMA
    # moves a contiguous 2KB line per partition.
    x8_d = nc.dram_tensor("x8", [P, 4, NCHUNK, 1024], f8, kind="ExternalInput")
    wq8_d = nc.dram_tensor("wq8", [P, NCHUNK, NCHUNK, P], f8, kind="ExternalInput")
    wu8_d = nc.dram_tensor("wu8", [P, NCHUNK, C], f8, kind="ExternalInput")
    out_d = nc.dram_tensor("out", [NCHUNK, P, NHALF], bf16, kind="ExternalOutput")
    den_d = nc.dram_tensor("den", [1, NHALF], f32, kind="ExternalOutput")

    def xj(x8t, jc):
        """lhsT pair [128, 2, 128] for j-chunk jc (columns jc*128..+128)."""
        return x8t[:, jc // 8, :, (jc % 8) * P : (jc % 8) * P + P]

    def xi(x8t, s):
        """rhs pair [128, 2, 512] for i-strip s (columns s*512..+512)."""
        return x8t[:, s // 2, :, (s % 2) * BLK : (s % 2) * BLK + BLK]

    with tile.TileContext(nc) as tc:
        with (
            tc.tile_pool(name="warm", bufs=1) as warm_pool,
            tc.tile_pool(name="xpool", bufs=1) as x_pool,
            tc.tile_pool(name="wpool", bufs=1) as w_pool,
            tc.tile_pool(name="rpool", bufs=1) as r_pool,
            tc.tile_pool(name="vpool", bufs=1) as v_pool,
            tc.tile_pool(name="eq", bufs=3) as eq_pool,
            tc.tile_pool(name="opool", bufs=3) as o_pool,
            tc.tile_pool(name="small", bufs=1) as s_pool,
            tc.tile_pool(name="ps_s", bufs=2, space="PSUM") as ps_s,
            tc.tile_pool(name="ps_av", bufs=1, space="PSUM") as ps_av,
            tc.tile_pool(name="ps_den", bufs=1, space="PSUM") as ps_den,
            tc.tile_pool(name="ps_vp", bufs=1, space="PSUM") as ps_vp,
        ):
            # ---- constants (gpsimd queue: memsets only, so they run first)
            wtile = warm_pool.tile([P, BLK], bf16, tag="warm")
            nc.vector.memset(wtile[:].bitcast(mybir.dt.uint16), 0)
            eb = s_pool.tile([P, 1], f32, tag="eb")
            nc.vector.memset(eb[:], EXPB)
            ones8 = s_pool.tile([P, NCHUNK, 16], f8, tag="ones8")
            nc.vector.memset(ones8[:].bitcast(u8), 0x38)  # fp8e4m3 1.0

            # ---- PE warmup: trip the HAM clock gate while DMAs stream ----
            for _ in range(NWARM):
                wps = ps_s.tile([P, NCHUNK, BLK], f32, tag="sp")
                nc.tensor.matmul(
                    wps[:, 0, :], wtile[:, 0:P], wtile[:], start=True, stop=True
                )

            # ---- input loads: 2 HW-DGE queues, first-needed first ----
            wq8 = w_pool.tile([P, NCHUNK, NCHUNK, P], f8, tag="wq8")
            nc.sync.dma_start(wq8[:], wq8_d.ap())
            wu8 = w_pool.tile([P, NCHUNK, C], f8, tag="wu8")
            nc.scalar.dma_start(wu8[:], wu8_d.ap())
            x8 = x_pool.tile([P, 4, NCHUNK, 1024], f8, tag="x8")
            for s in range(2):
                nc.sync.dma_start(x8[:, s, :, :], x8_d.ap()[:, s, :, :])
            for s in range(2, 4):
                nc.scalar.dma_start(x8[:, s, :, :], x8_d.ap()[:, s, :, :])

            r8 = r_pool.tile([P, NCHUNK, NHALF], f8, tag="r8")
            vt8 = v_pool.tile([P, NJC, C], f8, tag="vt8")

            # ---- r projection (8 DR matmuls) + vt pairs 0-3 upfront ----
            def emit_r_strip(s, split=False):
                rp = ps_s.tile([P, NCHUNK, BLK], f32, tag="sp")
                for b in range(NCHUNK):
                    nc.tensor.matmul(
                        rp[:, b, :],
                        wq8[:, :, b, :],
                        xi(x8, s),
                        start=True,
                        stop=True,
                        perf_mode=DR,
                    )
                sl = slice(s * BLK, (s + 1) * BLK)
                with nc.allow_low_precision(reason="fp8 r"):
                    if split:
                        nc.vector.tensor_scalar_mul(r8[:, 0, sl], rp[:, 0, :], 0.25)
                        nc.scalar.activation(r8[:, 1, sl], rp[:, 1, :], AF.Copy, scale=0.25)
                    else:
                        nc.vector.tensor_scalar_mul(r8[:, :, sl], rp[:], 0.25)

            def emit_vt_pair_mm(pair):
                vp = ps_vp.tile([P, NCHUNK, C], f32, tag="vp")
                for jj in range(2):
                    jc = 2 * pair + jj
                    nc.tensor.matmul(
                        vp[:, jj, :],
                        xj(x8, jc),
                        wu8[:],
                        start=True,
                        stop=True,
                        perf_mode=DR,
                    )
                return vp

            def emit_vt_pair_copy(pair, vp, eng="dve"):
                with nc.allow_low_precision(reason="fp8 vt"):
                    if eng == "act":
                        nc.scalar.activation(
                            vt8[:, 2 * pair : 2 * pair + 2, :],
                            vp[:],
                            AF.Copy,
                            scale=1 / 16.0,
                        )
                    else:
                        nc.vector.tensor_scalar_mul(
                            vt8[:, 2 * pair : 2 * pair + 2, :], vp[:], 1 / 16.0
                        )

            emit_r_strip(0, split=True)

            # ---- attention blocks ----
            avs = {}
            dens = {}

            def out_tail(blk, fast=False):
                # Wo is folded into the AV weights (Wu = Wo @ Wv'), so the av
                # accumulator IS the projected output: just copy + DMA.
                avb = avs.pop(blk)
                sl = slice(blk * BLK, (blk + 1) * BLK)
                ob = o_pool.tile([P, NCHUNK, BLK], bf16, tag="ob")
                with nc.allow_low_precision(reason="bf16 out"):
                    if fast:
                        nc.vector.tensor_copy(ob[:, 0, :], avb[:, 0, :])
                        nc.scalar.activation(ob[:, 1, :], avb[:, 1, :], AF.Copy)
                        nc.sync.dma_start(out_d.ap()[0, :, sl], ob[:, 0, :])
                        nc.scalar.dma_start(out_d.ap()[1, :, sl], ob[:, 1, :])
                    else:
                        nc.vector.tensor_copy(ob[:], avb[:])
                        nc.sync.dma_start(
                            out_d.ap().rearrange("a p n -> p a n")[:, :, sl], ob[:]
                        )

            den_sb = s_pool.tile([1, NHALF], f32, tag="den_sb")

            def den_tail(blk):
                denp = dens.pop(blk)
                nc.vector.tensor_copy(
                    den_sb[:, blk * BLK : (blk + 1) * BLK], denp[:]
                )
                if blk == NBLK - 1:
                    nc.scalar.dma_start(den_d.ap(), den_sb[:])

            NQ = NJC // QUART
            for blk in range(NBLK):
                ib = blk * BLK
                av = ps_av.tile([P, NCHUNK, BLK], f32, tag="av")
                denp = ps_den.tile([1, BLK], f32, tag="den")
                dens[blk] = denp
                eqs = {}
                # software pipeline: scores/exp for quarter q one step ahead
                # of AV/den for quarter q-1. During block 0 the remaining vT
                # projection pairs (4-15) are drizzled in two per quarter.
                for quart in range(NQ + 1):
                    if quart < NQ:
                        eq = eq_pool.tile([P, QUART, BLK], f8, tag="eq")
                        eqs[quart] = eq
                        for u in range(2):
                            sp = ps_s.tile([P, 2, BLK], f32, tag="sp")
                            for t in range(2):
                                jc = QUART * quart + 2 * u + t
                                nc.tensor.matmul(
                                    sp[:, t, :],
                                    xj(x8, jc),
                                    r8[:, :, ib : ib + BLK],
                                    start=True,
                                    stop=True,
                                    perf_mode=DR,
                                )
                            with nc.allow_low_precision(reason="fp8 exp"):
                                nc.scalar.activation(
                                    eq[:, 2 * u : 2 * u + 2, :],
                                    sp[:],
                                    AF.Exp,
                                    bias=eb[:],
                                    scale=SCALE,
                                )
                    # block 0 streams the vT projection: pairs 0-3 burst in
                    # quarter 0 (copies alternate DVE/ACT while ACT idles
                    # during pipeline priming), pairs (2q+2, 2q+3) inside
                    # quarter q afterwards; AV needs a pair a quarter later.
                    if blk == 0 and quart == 0:
                        for pair in range(4):
                            vp0 = emit_vt_pair_mm(pair)
                            emit_vt_pair_copy(pair, vp0, "act" if pair % 2 else "dve")
                    elif blk == 0 and 1 <= quart <= 6:
                        vp0 = emit_vt_pair_mm(2 * quart + 2)
                        emit_vt_pair_copy(2 * quart + 2, vp0)
                    if quart == 5 and blk < NBLK - 1:
                        emit_r_strip(blk + 1)
                    if quart > 0:
                        q0 = quart - 1
                        eq = eqs.pop(q0)
                        for u in range(2):
                            pr = 2 * q0 + u  # pair index 0..15
                            jc0 = QUART * q0 + 2 * u
                            for m in range(NCHUNK):
                                nc.tensor.matmul(
                                    av[:, m, :],
                                    vt8[:, jc0 : jc0 + 2, m * P : (m + 1) * P],
                                    eq[:, 2 * u : 2 * u + 2, :],
                                    start=(pr == 0),
                                    stop=(pr == 15),
                                    perf_mode=DR,
                                )
                            nc.tensor.matmul(
                                denp[:],
                                ones8[:, :, 0:1],
                                eq[:, 2 * u : 2 * u + 2, :],
                                start=(pr == 0),
                                stop=(pr == 15),
                                perf_mode=DR,
                            )
                            if u == 0 and blk == 0 and 1 <= quart <= 6:
                                vp1 = emit_vt_pair_mm(2 * quart + 3)
                                emit_vt_pair_copy(2 * quart + 3, vp1)


                den_tail(blk)
                avs[blk] = av
                out_tail(blk, fast=(blk == NBLK - 1))

    nc.compile()
    return nc


def _prep_shards(x, gamma, beta, Wq, bq, Wk, bk, Wv, bv, Wo, bo):
    import ml_dtypes

    E4 = ml_dtypes.float8_e4m3

    xr = np.ascontiguousarray(x, dtype=np.float32).reshape(4, C, N)
    gamma = np.asarray(gamma, np.float64)
    beta = np.asarray(beta, np.float64)
    Wq64 = np.asarray(Wq, np.float64)
    Wk64 = np.asarray(Wk, np.float64)
    Wv64 = np.asarray(Wv, np.float64)
    Wo64 = np.asarray(Wo, np.float64)

    def w4(W):
        # w4[p, a, b, m] = W[b*128+m, a*128+p]
        return np.ascontiguousarray(
            np.asarray(W, np.float32)
            .reshape(NCHUNK, P, NCHUNK, P)
            .transpose(3, 2, 0, 1)
            .astype(E4)
        )

    def wv3(W):
        return np.ascontiguousarray(
            np.asarray(W, np.float32).reshape(C, NCHUNK, P).transpose(2, 1, 0).astype(E4)
        )

    in_maps = []
    add_c = []
    per_img = {}
    for core in range(8):
        img = core // 2
        if core % 2 == 0:
            xi = xr[img]  # [C, N]
            xg = xi.reshape(NG, GS * N).astype(np.float64)
            mean = xg.mean(axis=1)
            var = xg.var(axis=1)
            rstd = 1.0 / np.sqrt(var + EPS)
            scale_c = gamma * np.repeat(rstd, GS)
            shift_c = beta - np.repeat(mean, GS) * scale_c
            Wqp = Wq64 * scale_c[None, :]
            Wkp = Wk64 * scale_c[None, :]
            M = Wqp.T @ Wkp
            bvrow = np.asarray(bv, np.float64) + Wv64 @ shift_c
            add_c.append(Wo64 @ bvrow + np.asarray(bo, np.float64))
            xc = xi.reshape(NCHUNK, P, N).transpose(1, 0, 2)  # [P, 2, N]
            Wu = Wo64 @ (Wv64 * scale_c[None, :])
            per_img = {
                "wq8": w4(4.0 * M.T),
                "wu8": wv3(16.0 * Wu),
                "x": np.ascontiguousarray(xc),
            }
        xc = per_img["x"]
        if core % 2 == 1:
            xc = np.roll(xc, -NHALF, axis=2)
        # strip-major fp8: [P, strip, chunk, 1024]
        x8 = np.ascontiguousarray(
            xc.reshape(P, NCHUNK, 4, 1024).transpose(0, 2, 1, 3).astype(E4)
        )
        m = {k: v for k, v in per_img.items() if k != "x"}
        m["x8"] = x8
        in_maps.append(m)
    return in_maps, np.asarray(add_c, np.float64)


def kernel(x, gamma, beta, Wq, bq, Wk, bk, Wv, bv, Wo, bo, _trace=False):
    from concourse.bass_utils import run_bass_kernel_spmd

    if "nc" not in _CACHE:
        _CACHE["nc"] = _build_program()
    nc = _CACHE["nc"]

    in_maps, add_c = _prep_shards(x, gamma, beta, Wq, bq, Wk, bk, Wv, bv, Wo, bo)
    res = run_bass_kernel_spmd(nc, in_maps, core_ids=list(range(8)), trace=_trace)
    _CACHE["last_results"] = res

    x_np = np.ascontiguousarray(x, dtype=np.float32).reshape(4, C, N)
    y = np.empty((4, C, N), np.float32)
    for core in range(8):
        o = res.results[core]["out"].astype(np.float32).reshape(C, NHALF)
        den = res.results[core]["den"].astype(np.float32).reshape(1, NHALF)
        img = core // 2
        lo, hi = (0, NHALF) if core % 2 == 0 else (NHALF, N)
        y[img, :, lo:hi] = (
            x_np[img, :, lo:hi] + o / den + add_c[img].astype(np.float32)[:, None]
        )
    return y.reshape(4, C, 64, 64)
